# revision 26
# baseline (speedup 1.0000x reference)
"""BitNetLinear Trainium2 kernel (8 NeuronCores, SPMD data-parallel).

y = round(clip(x, +-127*s)/s)*s @ (ternary(W))^T + ternary(b)
with s = exp2(floor(log2(max|x|/127 + eps))) a power of two (global over x).

Sharding: batch dim (8) -> one batch of [4096, 1024] per core.
Host prep: x shard transposed to [in, rows] (PE contracts over partitions);
weight/bias ternary-quantized on host (reference does this in __init__);
ternary weight shipped as bf16 [in, out].

Device: phase 1 streams x computing the local absmax. Because
floor(log2(.)) commutes with max, the global power-of-two scale equals the
max of the per-core local scales, so each core SPECULATES with its local
scale and starts quantize+matmul immediately while a 512B AllReduce(max)
flies concurrently on the collectives hardware. At the end each core
compares its speculative scale against the global one and, on mismatch,
re-runs the (exact) quantize+matmul with the global scale under a
conditional branch. x is quantized to integer-valued bf16 (round-half-even
via the +-1.5*2^23 trick); the bf16 matmul with fp32 PSUM accumulation is
exact integer arithmetic (|x_int| <= 127, w in {-1,0,1}, |acc| < 2^24);
the result is scaled by s*gamma_w and the ternary bias is added.
"""

import numpy as np
import ml_dtypes
from contextlib import ExitStack

import concourse.bass as bass
import concourse.mybir as mybir
import concourse.tile as tile
from concourse import bacc, bass_isa, bass_utils

F32 = mybir.dt.float32
BF16 = mybir.dt.bfloat16
I32 = mybir.dt.int32

N_CORES = 8
P = 128
IN_F = 1024
OUT_F = 1024
KC = IN_F // P          # 8 contraction chunks
RSUB = 256              # rows quantized per chunk
ROUND_C = 12582912.0    # 1.5 * 2**23: float32 round-half-even trick
EPS = 1e-8


def _emit_scale_chain(nc, consts, gmax, gamma_sb, mask_t, expc_t, tag,
                      eng=None):
    """From a [P,1] absmax tile, compute s = exp2(floor(log2(m/127+eps)))
    via exponent masking, 1/s via exponent arithmetic, and c = s*gamma."""
    if eng is None:
        eng = nc.vector
    v_t = consts.tile([P, 1], F32, tag=f"v_{tag}")
    eng.tensor_scalar(
        out=v_t,
        in0=gmax,
        scalar1=float(np.float32(1.0 / 127.0)),
        scalar2=float(np.float32(EPS)),
        op0=mybir.AluOpType.mult,
        op1=mybir.AluOpType.add,
    )
    s_t = consts.tile([P, 1], F32, tag=f"s_{tag}")
    eng.tensor_tensor(
        out=s_t.bitcast(I32),
        in0=v_t.bitcast(I32),
        in1=mask_t,
        op=mybir.AluOpType.bitwise_and,
    )
    inv_t = consts.tile([P, 1], F32, tag=f"inv_{tag}")
    eng.tensor_tensor(
        out=inv_t.bitcast(I32),
        in0=expc_t,
        in1=s_t.bitcast(I32),
        op=mybir.AluOpType.subtract,
    )
    c_t = consts.tile([P, 1], F32, tag=f"c_{tag}")
    eng.tensor_mul(out=c_t, in0=s_t, in1=gamma_sb)
    return s_t, inv_t, c_t


def _emit_phase2(nc, pools, rows, xt_cols, y_rows, w_sb, bias_sb, negc_t,
                 inv_t, c_t):
    """Quantize x with 1/s, matmul against the ternary weight, scale by c,
    add bias, store y."""
    xq_pool, tq_pool, ub_pool, xi_pool, yo_pool, ps_pool = pools
    nhalf = OUT_F // 512
    for t in range(rows // RSUB):
        xc = xq_pool.tile([P, KC, RSUB], F32, tag="xc")
        nc.sync.dma_start(out=xc, in_=xt_cols[t])
        tq = tq_pool.tile([P, KC, RSUB], F32, tag="tq")
        nc.vector.tensor_scalar(
            out=tq,
            in0=xc,
            scalar1=inv_t,
            scalar2=ROUND_C,
            op0=mybir.AluOpType.mult,
            op1=mybir.AluOpType.add,
        )
        ub = ub_pool.tile([P, KC, RSUB], BF16, tag="ub")
        nc.scalar.activation(
            out=ub,
            in_=tq,
            func=mybir.ActivationFunctionType.Identity,
            bias=negc_t,
            scale=1.0,
        )
        xi = xi_pool.tile([P, KC, RSUB], BF16, tag="xi")
        nc.vector.tensor_scalar(
            out=xi,
            in0=ub,
            scalar1=-127.0,
            scalar2=127.0,
            op0=mybir.AluOpType.max,
            op1=mybir.AluOpType.min,
        )
        for h in range(RSUB // P):
            ps = ps_pool.tile([P, OUT_F], F32, tag="ps")
            for k in range(KC):
                for n in range(nhalf):
                    nc.tensor.matmul(
                        ps[:, n * 512 : (n + 1) * 512],
                        lhsT=xi[:, k, h * P : (h + 1) * P],
                        rhs=w_sb[:, k, n * 512 : (n + 1) * 512],
                        start=(k == 0),
                        stop=(k == KC - 1),
                    )
            yo = yo_pool.tile([P, OUT_F], F32, tag="yo")
            nc.scalar.activation(
                out=yo,
                in_=ps,
                func=mybir.ActivationFunctionType.Copy,
                bias=0.0,
                scale=c_t,
            )
            nc.vector.tensor_add(out=yo, in0=yo, in1=bias_sb)
            nc.sync.dma_start(out=y_rows[t * (RSUB // P) + h], in_=yo)


def build_program(rows: int = 4096, num_cores: int = N_CORES,
                  speculate: bool = True) -> bacc.Bacc:
    assert rows % RSUB == 0
    nc = bacc.Bacc(
        "TRN2",
        target_bir_lowering=False,
        debug=False,
        enable_asserts=False,
        num_devices=num_cores,
    )
    nt = rows // RSUB
    # x shard pre-tiled on host: xt[t, p, c, r] = x[t*RSUB + r, c*P + p],
    # so every phase-2 chunk load is one fully-contiguous 512 KiB read.
    xt = nc.dram_tensor("xt", (nt, P, KC, RSUB), F32, kind="ExternalInput").ap()
    wq = nc.dram_tensor("wq", (IN_F, OUT_F), BF16, kind="ExternalInput").ap()
    bq = nc.dram_tensor("bq", (1, OUT_F), F32, kind="ExternalInput").ap()
    gq = nc.dram_tensor("gq", (1, 1), F32, kind="ExternalInput").ap()
    y = nc.dram_tensor("y", (rows, OUT_F), F32, kind="ExternalOutput").ap()
    # Collectives cannot target I/O tensors; bounce through internal DRAM.
    cc_in = nc.dram_tensor("cc_in", (P, 1), F32).ap()
    cc_out = nc.dram_tensor("cc_out", (P, 1), F32).ap()

    with tile.TileContext(nc, num_cores=num_cores) as tc, ExitStack() as ctx:
        consts = ctx.enter_context(tc.tile_pool(name="consts", bufs=1))

        mask_t = consts.tile([P, 1], I32)
        nc.vector.memset(mask_t, -8388608)  # 0xFF800000: sign+exponent mask
        expc_t = consts.tile([P, 1], I32)
        nc.vector.memset(expc_t, 0x7F000000)  # bits of (254<<23)
        negc_t = consts.tile([P, 1], F32)
        nc.vector.memset(negc_t, -ROUND_C)

        # --- phase 1: local absmax of the x shard (two t-chunks per DMA
        # for ~2 MiB transfers) ---
        g = 2 if nt % 2 == 0 else 1
        xt_pairs = xt.rearrange("(j g) p c r -> j p g c r", g=g)
        partials = consts.tile([P, nt // g], F32)
        with tc.tile_pool(name="xmax", bufs=4) as xpool:
            for j in range(nt // g):
                xsb = xpool.tile([P, g, KC, RSUB], F32)
                nc.sync.dma_start(out=xsb, in_=xt_pairs[j])
                nc.vector.tensor_reduce(
                    out=partials[:, j : j + 1],
                    in_=xsb,
                    axis=mybir.AxisListType.XYZ,
                    op=mybir.AluOpType.max,
                    apply_absolute_value=True,
                )
        lmax = consts.tile([P, 1], F32)
        nc.vector.tensor_reduce(
            out=lmax, in_=partials, axis=mybir.AxisListType.X,
            op=mybir.AluOpType.max,
        )
        gmax_l = consts.tile([P, 1], F32)
        nc.gpsimd.partition_all_reduce(
            gmax_l, lmax, channels=P, reduce_op=bass_isa.ReduceOp.max
        )

        # --- global max across the 8 cores (gpsimd queue so the bounce
        # DMAs never block the sync queue that feeds phase 2) ---
        nc.gpsimd.dma_start(out=cc_in, in_=gmax_l)
        nc.gpsimd.collective_compute(
            "AllReduce",
            mybir.AluOpType.max,
            replica_groups=[list(range(num_cores))],
            ins=[cc_in.opt()],
            outs=[cc_out.opt()],
        )
        gmax_g = consts.tile([P, 1], F32)
        nc.gpsimd.dma_start(out=gmax_g, in_=cc_out)

        # --- constants: ternary weight [p, kc, o], bias row, gamma scalar.
        # Issued on the ACT HWDGE ring so they never sit between phase-1
        # and phase-2 loads in the sync-ring FIFO. ---
        w_sb = consts.tile([P, KC, OUT_F], BF16)
        nc.scalar.dma_start(out=w_sb, in_=wq.rearrange("(c p) o -> p c o", p=P))
        bias_sb = consts.tile([P, OUT_F], F32)
        nc.scalar.dma_start(out=bias_sb, in_=bq.to_broadcast((P, OUT_F)))
        gamma_sb = consts.tile([P, 1], F32)
        nc.scalar.dma_start(out=gamma_sb, in_=gq.to_broadcast((P, 1)))

        # --- PE warmup: junk matmuls gated on the local max; they flip HAM
        # to full clock while the scale chain + first quantize run ---
        warm_rhs = consts.tile([P, 512], BF16)
        nc.vector.memset(warm_rhs, 0.0)
        nc.vector.tensor_copy(out=warm_rhs[:, 0:1], in_=gmax_l)
        with tc.tile_pool(name="warm_ps", bufs=1, space="PSUM") as warm_pool:
            warm_ps = warm_pool.tile([P, 512], F32)
            for _ in range(10):
                nc.tensor.matmul(
                    warm_ps, lhsT=w_sb[:, 0, 0:P], rhs=warm_rhs,
                    start=True, stop=True,
                )

        xt_cols = xt
        y_rows = y.rearrange("(t p) o -> t p o", p=P)

        with (
            tc.tile_pool(name="xq", bufs=6) as xq_pool,
            tc.tile_pool(name="tq", bufs=3) as tq_pool,
            tc.tile_pool(name="ub", bufs=3) as ub_pool,
            tc.tile_pool(name="xi", bufs=4) as xi_pool,
            tc.tile_pool(name="yo", bufs=4) as yo_pool,
            tc.tile_pool(name="ps", bufs=4, space="PSUM") as ps_pool,
        ):
            pools = (xq_pool, tq_pool, ub_pool, xi_pool, yo_pool, ps_pool)
            if not speculate:
                _, inv_g, c_g = _emit_scale_chain(
                    nc, consts, gmax_g, gamma_sb, mask_t, expc_t, "g")
                _emit_phase2(nc, pools, rows, xt_cols, y_rows, w_sb, bias_sb,
                             negc_t, inv_g, c_g)
            else:
                s_l, inv_l, c_l = _emit_scale_chain(
                    nc, consts, gmax_l, gamma_sb, mask_t, expc_t, "l")
                twos_l = consts.tile([P, 1], F32)
                nc.vector.tensor_scalar(
                    out=twos_l, in0=s_l, scalar1=2.0, scalar2=None,
                    op0=mybir.AluOpType.mult,
                )
                _emit_phase2(nc, pools, rows, xt_cols, y_rows, w_sb, bias_sb,
                             negc_t, inv_l, c_l)

                # --- verify the speculation: the local scale is global iff
                # v_g = gmax_g/127+eps stays in the same binade, i.e.
                # v_g - 2*s_l < 0. Computed on the idle gpsimd engine so the
                # check (which waits on the AllReduce) never stalls the DVE
                # pipeline mid-phase. ---
                v_g2 = consts.tile([P, 1], F32)
                nc.gpsimd.tensor_scalar(
                    out=v_g2,
                    in0=gmax_g,
                    scalar1=float(np.float32(1.0 / 127.0)),
                    scalar2=float(np.float32(EPS)),
                    op0=mybir.AluOpType.mult,
                    op1=mybir.AluOpType.add,
                )
                chk = consts.tile([P, 1], F32)
                nc.gpsimd.tensor_tensor(
                    out=chk, in0=v_g2, in1=twos_l, op=mybir.AluOpType.subtract
                )
                regs = nc.alloc_registers(
                    "spec_chk",
                    bass.OrderedSet([
                        mybir.EngineType.SP,
                        mybir.EngineType.DVE,
                        mybir.EngineType.Activation,
                        mybir.EngineType.PE,
                    ]),
                )
                for reg in regs:
                    nc.reg_load(reg, chk.bitcast(I32)[0:1, 0:1])
                # f32 bits as int32: negative iff chk < 0 (fast path)
                with tc.If(nc.snap(regs) >= 0):
                    # mismatch: redo everything with the global scale
                    _, inv_g, c_g = _emit_scale_chain(
                        nc, consts, gmax_g, gamma_sb, mask_t, expc_t, "g")
                    _emit_phase2(nc, pools, rows, xt_cols, y_rows, w_sb,
                                 bias_sb, negc_t, inv_g, c_g)

    nc.compile()
    return nc


def quantize_params(weight: np.ndarray, bias: np.ndarray):
    """Ternary-quantize weight/bias exactly as the reference (f64 math whose
    f32 rounding matches jax-f32; verified margins are orders of magnitude
    above f32 accumulation differences)."""
    w64 = weight.astype(np.float64)
    g_w = np.float32(np.abs(w64).mean())
    wi = np.clip(np.round(w64 / (np.float64(g_w) + EPS)), -1.0, 1.0)
    b64 = bias.astype(np.float64)
    g_b = np.float32(np.abs(b64).mean())
    bi = np.clip(np.round(b64 / (np.float64(g_b) + EPS)), -1.0, 1.0)
    bq = (bi * np.float64(g_b)).astype(np.float32)  # exact: {-g_b, 0, g_b}
    return wi, g_w, bq


_PROGRAM_CACHE: dict[int, bacc.Bacc] = {}


def _get_program(rows: int) -> bacc.Bacc:
    if rows not in _PROGRAM_CACHE:
        _PROGRAM_CACHE[rows] = build_program(rows)
    return _PROGRAM_CACHE[rows]


def tile_x_shard(x2d: np.ndarray) -> np.ndarray:
    """[rows, IN_F] -> [nt, P, KC, RSUB] with xt[t,p,c,r] = x[t*RSUB+r, c*P+p]."""
    rows = x2d.shape[0]
    return np.ascontiguousarray(
        x2d.reshape(rows // RSUB, RSUB, KC, P).transpose(0, 3, 2, 1)
    )


def prepare_in_maps(x: np.ndarray, weight: np.ndarray, bias: np.ndarray):
    x = np.asarray(x, dtype=np.float32)
    weight = np.asarray(weight, dtype=np.float32)
    bias = np.asarray(bias, dtype=np.float32)
    batch, rows, in_f = x.shape
    assert batch == N_CORES and in_f == IN_F and weight.shape == (OUT_F, IN_F)

    wi, g_w, bq = quantize_params(weight, bias)
    wq_t = np.ascontiguousarray(wi.T).astype(ml_dtypes.bfloat16)  # [in, out]
    bq_row = np.ascontiguousarray(bq.reshape(1, OUT_F))
    gq = np.array([[g_w]], dtype=np.float32)

    in_maps = []
    for c in range(N_CORES):
        in_maps.append(
            {
                "xt": tile_x_shard(x[c]),
                "wq": wq_t,
                "bq": bq_row,
                "gq": gq,
            }
        )
    return in_maps, rows


def kernel(x: np.ndarray, weight: np.ndarray, bias: np.ndarray) -> np.ndarray:
    in_maps, rows = prepare_in_maps(x, weight, bias)
    nc = _get_program(rows)
    res = bass_utils.run_bass_kernel_spmd(nc, in_maps, core_ids=list(range(N_CORES)))
    return np.stack([res.results[c]["y"] for c in range(N_CORES)], axis=0)


# revision 34
# speedup vs baseline: 1.1368x; 1.1368x over previous
"""BitNetLinear Trainium2 kernel (8 NeuronCores, SPMD data-parallel).

y = round(clip(x, +-127*s)/s)*s @ (ternary(W))^T + ternary(b)
with s = exp2(floor(log2(max|x|/127 + eps))) a power of two (global over x).

Sharding: batch dim (8) -> one batch of [4096, 1024] per core.
Host prep: x shard transposed to [in, rows] (PE contracts over partitions);
weight/bias ternary-quantized on host (reference does this in __init__);
ternary weight shipped as bf16 [in, out].

Device: phase 1 streams x computing the local absmax. Because
floor(log2(.)) commutes with max, the global power-of-two scale equals the
max of the per-core local scales, so each core SPECULATES with its local
scale and starts quantize+matmul immediately while a 512B AllReduce(max)
flies concurrently on the collectives hardware. At the end each core
compares its speculative scale against the global one and, on mismatch,
re-runs the (exact) quantize+matmul with the global scale under a
conditional branch. x is quantized to integer-valued bf16 (round-half-even
via the +-1.5*2^23 trick); the bf16 matmul with fp32 PSUM accumulation is
exact integer arithmetic (|x_int| <= 127, w in {-1,0,1}, |acc| < 2^24);
the result is scaled by s*gamma_w and the ternary bias is added.
"""

import numpy as np
import ml_dtypes
from contextlib import ExitStack

import concourse.bass as bass
import concourse.mybir as mybir
import concourse.tile as tile
from concourse import bacc, bass_isa, bass_utils

F32 = mybir.dt.float32
BF16 = mybir.dt.bfloat16
I32 = mybir.dt.int32

N_CORES = 8
P = 128
IN_F = 1024
OUT_F = 1024
KC = IN_F // P          # 8 contraction chunks
RSUB = 256              # rows quantized per chunk
ROUND_C = 12582912.0    # 1.5 * 2**23: float32 round-half-even trick
EPS = 1e-8


def _emit_scale_chain(nc, consts, gmax, gamma_sb, mask_t, expc_t, tag,
                      eng=None):
    """From a [P,1] absmax tile, compute s = exp2(floor(log2(m/127+eps)))
    via exponent masking, 1/s via exponent arithmetic, and c = s*gamma."""
    if eng is None:
        eng = nc.vector
    v_t = consts.tile([P, 1], F32, tag=f"v_{tag}")
    eng.tensor_scalar(
        out=v_t,
        in0=gmax,
        scalar1=float(np.float32(1.0 / 127.0)),
        scalar2=float(np.float32(EPS)),
        op0=mybir.AluOpType.mult,
        op1=mybir.AluOpType.add,
    )
    s_t = consts.tile([P, 1], F32, tag=f"s_{tag}")
    eng.tensor_tensor(
        out=s_t.bitcast(I32),
        in0=v_t.bitcast(I32),
        in1=mask_t,
        op=mybir.AluOpType.bitwise_and,
    )
    inv_t = consts.tile([P, 1], F32, tag=f"inv_{tag}")
    eng.tensor_tensor(
        out=inv_t.bitcast(I32),
        in0=expc_t,
        in1=s_t.bitcast(I32),
        op=mybir.AluOpType.subtract,
    )
    c_t = consts.tile([P, 1], F32, tag=f"c_{tag}")
    eng.tensor_mul(out=c_t, in0=s_t, in1=gamma_sb)
    return s_t, inv_t, c_t


def _emit_phase2(nc, pools, rows, xt_cols, y_rows, w_sb, bias_sb, negc_t,
                 inv_t, c_t, const_loader=None):
    """Quantize x with 1/s, matmul against the ternary weight, scale by c,
    add bias, store y. Returns the last emitted instruction per engine."""
    xq_pool, tq_pool, ub_pool, xi_pool, yo_pool, ps_pool = pools
    nhalf = OUT_F // 512
    last = {}
    for t in range(rows // RSUB):
        xc = xq_pool.tile([P, KC, RSUB], F32, tag="xc")
        last["SP"] = nc.sync.dma_start(out=xc, in_=xt_cols[t])
        if t == 0 and const_loader is not None:
            const_loader()
        tq = tq_pool.tile([P, KC, RSUB], F32, tag="tq")
        nc.vector.tensor_scalar(
            out=tq,
            in0=xc,
            scalar1=inv_t,
            scalar2=ROUND_C,
            op0=mybir.AluOpType.mult,
            op1=mybir.AluOpType.add,
        )
        ub = ub_pool.tile([P, KC, RSUB], BF16, tag="ub")
        nc.scalar.activation(
            out=ub,
            in_=tq,
            func=mybir.ActivationFunctionType.Identity,
            bias=negc_t,
            scale=1.0,
        )
        xi = xi_pool.tile([P, KC, RSUB], BF16, tag="xi")
        nc.vector.tensor_scalar(
            out=xi,
            in0=ub,
            scalar1=-127.0,
            scalar2=127.0,
            op0=mybir.AluOpType.max,
            op1=mybir.AluOpType.min,
        )
        for h in range(RSUB // P):
            ps = ps_pool.tile([P, OUT_F], F32, tag="ps")
            for k in range(KC):
                for n in range(nhalf):
                    last["PE"] = nc.tensor.matmul(
                        ps[:, n * 512 : (n + 1) * 512],
                        lhsT=xi[:, k, h * P : (h + 1) * P],
                        rhs=w_sb[:, k, n * 512 : (n + 1) * 512],
                        start=(k == 0),
                        stop=(k == KC - 1),
                    )
            yo = yo_pool.tile([P, OUT_F], F32, tag="yo")
            last["ACT"] = nc.scalar.activation(
                out=yo,
                in_=ps,
                func=mybir.ActivationFunctionType.Copy,
                bias=0.0,
                scale=c_t,
            )
            last["DVE"] = nc.vector.tensor_add(out=yo, in0=yo, in1=bias_sb)
            last["SP"] = nc.sync.dma_start(out=y_rows[t * (RSUB // P) + h], in_=yo)
    return last


def build_program(rows: int = 4096, num_cores: int = N_CORES,
                  speculate: bool = True) -> bacc.Bacc:
    assert rows % RSUB == 0
    nc = bacc.Bacc(
        "TRN2",
        target_bir_lowering=False,
        debug=False,
        enable_asserts=False,
        num_devices=num_cores,
    )
    nt = rows // RSUB
    # x shard pre-tiled on host: xt[t, p, c, r] = x[t*RSUB + r, c*P + p],
    # so every phase-2 chunk load is one fully-contiguous 512 KiB read.
    xt = nc.dram_tensor("xt", (nt, P, KC, RSUB), F32, kind="ExternalInput").ap()
    wq = nc.dram_tensor("wq", (IN_F, OUT_F), BF16, kind="ExternalInput").ap()
    bq = nc.dram_tensor("bq", (1, OUT_F), F32, kind="ExternalInput").ap()
    gq = nc.dram_tensor("gq", (1, 1), F32, kind="ExternalInput").ap()
    y = nc.dram_tensor("y", (rows, OUT_F), F32, kind="ExternalOutput").ap()
    # Collectives cannot target I/O tensors; bounce through internal DRAM.
    cc_in = nc.dram_tensor("cc_in", (P, 1), F32).ap()
    cc_out = nc.dram_tensor("cc_out", (P, 1), F32).ap()

    with tile.TileContext(nc, num_cores=num_cores) as tc, ExitStack() as ctx:
        consts = ctx.enter_context(tc.tile_pool(name="consts", bufs=1))

        mask_t = consts.tile([P, 1], I32)
        nc.vector.memset(mask_t, -8388608)  # 0xFF800000: sign+exponent mask
        expc_t = consts.tile([P, 1], I32)
        nc.vector.memset(expc_t, 0x7F000000)  # bits of (254<<23)
        negc_t = consts.tile([P, 1], F32)
        nc.vector.memset(negc_t, -ROUND_C)

        # --- phase 1: local absmax of the x shard (two t-chunks per DMA
        # for ~2 MiB transfers) ---
        g = 2 if nt % 2 == 0 else 1
        xt_pairs = xt.rearrange("(j g) p c r -> j p g c r", g=g)
        partials = consts.tile([P, nt // g], F32)
        with tc.tile_pool(name="xmax", bufs=4) as xpool:
            for j in range(nt // g):
                xsb = xpool.tile([P, g, KC, RSUB], F32)
                nc.sync.dma_start(out=xsb, in_=xt_pairs[j])
                nc.vector.tensor_reduce(
                    out=partials[:, j : j + 1],
                    in_=xsb,
                    axis=mybir.AxisListType.XYZ,
                    op=mybir.AluOpType.max,
                    apply_absolute_value=True,
                )
        lmax = consts.tile([P, 1], F32)
        nc.vector.tensor_reduce(
            out=lmax, in_=partials, axis=mybir.AxisListType.X,
            op=mybir.AluOpType.max,
        )
        gmax_l = consts.tile([P, 1], F32)
        nc.gpsimd.partition_all_reduce(
            gmax_l, lmax, channels=P, reduce_op=bass_isa.ReduceOp.max
        )

        # --- global max across the 8 cores (gpsimd queue so the bounce
        # DMAs never block the sync queue that feeds phase 2) ---
        nc.gpsimd.dma_start(out=cc_in, in_=gmax_l)
        nc.gpsimd.collective_compute(
            "AllReduce",
            mybir.AluOpType.max,
            replica_groups=[list(range(num_cores))],
            ins=[cc_in.opt()],
            outs=[cc_out.opt()],
        )
        gmax_g = consts.tile([P, 1], F32)
        nc.gpsimd.dma_start(out=gmax_g, in_=cc_out)

        # --- constants: ternary weight [p, kc, o], bias row, gamma scalar.
        # gamma early (the scale chain needs it); weight/bias injected into
        # the sync ring after the first two phase-2 x loads so the first
        # quantize starts as early as possible.
        gamma_sb = consts.tile([P, 1], F32)
        nc.sync.dma_start(out=gamma_sb, in_=gq.to_broadcast((P, 1)))
        w_sb = consts.tile([P, KC, OUT_F], BF16)
        bias_sb = consts.tile([P, OUT_F], F32)

        def const_loader():
            nc.sync.dma_start(
                out=w_sb, in_=wq.rearrange("(c p) o -> p c o", p=P))
            nc.sync.dma_start(out=bias_sb, in_=bq.to_broadcast((P, OUT_F)))

        # --- PE warmup: junk matmuls gated on the local max; they flip HAM
        # to full clock while the scale chain + first quantize run ---
        warm_rhs = consts.tile([P, 512], BF16)
        nc.vector.memset(warm_rhs, 0.0)
        nc.vector.tensor_copy(out=warm_rhs[:, 0:1], in_=gmax_l)
        with tc.tile_pool(name="warm_ps", bufs=1, space="PSUM") as warm_pool:
            warm_ps = warm_pool.tile([P, 512], F32)
            for _ in range(10):
                nc.tensor.matmul(
                    warm_ps, lhsT=warm_rhs[:, 0:P], rhs=warm_rhs,
                    start=True, stop=True,
                )

        xt_cols = xt
        y_rows = y.rearrange("(t p) o -> t p o", p=P)

        with (
            tc.tile_pool(name="xq", bufs=6) as xq_pool,
            tc.tile_pool(name="tq", bufs=3) as tq_pool,
            tc.tile_pool(name="ub", bufs=3) as ub_pool,
            tc.tile_pool(name="xi", bufs=4) as xi_pool,
            tc.tile_pool(name="yo", bufs=4) as yo_pool,
            tc.tile_pool(name="ps", bufs=4, space="PSUM") as ps_pool,
        ):
            pools = (xq_pool, tq_pool, ub_pool, xi_pool, yo_pool, ps_pool)
            if not speculate:
                _, inv_g, c_g = _emit_scale_chain(
                    nc, consts, gmax_g, gamma_sb, mask_t, expc_t, "g")
                _emit_phase2(nc, pools, rows, xt_cols, y_rows, w_sb, bias_sb,
                             negc_t, inv_g, c_g, const_loader=const_loader)
            else:
                s_l, inv_l, c_l = _emit_scale_chain(
                    nc, consts, gmax_l, gamma_sb, mask_t, expc_t, "l")
                twos_l = consts.tile([P, 1], F32)
                nc.vector.tensor_scalar(
                    out=twos_l, in0=s_l, scalar1=2.0, scalar2=None,
                    op0=mybir.AluOpType.mult,
                )
                last = _emit_phase2(nc, pools, rows, xt_cols, y_rows, w_sb,
                                    bias_sb, negc_t, inv_l, c_l,
                                    const_loader=const_loader)

                # --- verify the speculation: the local scale is global iff
                # v_g = gmax_g/127+eps stays in the same binade, i.e.
                # v_g - 2*s_l < 0. Computed on the idle gpsimd engine so the
                # check (which waits on the AllReduce) never stalls the DVE
                # pipeline mid-phase. ---
                v_g2 = consts.tile([P, 1], F32)
                nc.gpsimd.tensor_scalar(
                    out=v_g2,
                    in0=gmax_g,
                    scalar1=float(np.float32(1.0 / 127.0)),
                    scalar2=float(np.float32(EPS)),
                    op0=mybir.AluOpType.mult,
                    op1=mybir.AluOpType.add,
                )
                chk = consts.tile([P, 1], F32)
                nc.gpsimd.tensor_tensor(
                    out=chk, in0=v_g2, in1=twos_l, op=mybir.AluOpType.subtract
                )
                regs = nc.alloc_registers(
                    "spec_chk",
                    bass.OrderedSet([
                        mybir.EngineType.SP,
                        mybir.EngineType.DVE,
                        mybir.EngineType.Activation,
                        mybir.EngineType.PE,
                    ]),
                )
                # Pin each engine's reg_load after its last speculative-phase
                # instruction: the load waits on the AllReduce, and the Tile
                # scheduler would otherwise be free to place it mid-stream,
                # stalling that engine's FIFO on the collective.
                eng_key = {
                    mybir.EngineType.PE: "PE",
                    mybir.EngineType.DVE: "DVE",
                    mybir.EngineType.Activation: "ACT",
                    mybir.EngineType.SP: "SP",
                }
                for reg in regs:
                    ld = nc.reg_load(reg, chk.bitcast(I32)[0:1, 0:1])
                    prev = last.get(eng_key[reg.engine])
                    if prev is not None:
                        tile.add_dep_helper(
                            ld.ins, prev.ins, sync=False,
                            reason="speculation check after spec phase",
                        )
                # f32 bits as int32: negative iff chk < 0 (fast path)
                with tc.If(nc.snap(regs) >= 0):
                    # mismatch: redo everything with the global scale
                    _, inv_g, c_g = _emit_scale_chain(
                        nc, consts, gmax_g, gamma_sb, mask_t, expc_t, "g")
                    _emit_phase2(nc, pools, rows, xt_cols, y_rows, w_sb,
                                 bias_sb, negc_t, inv_g, c_g)

    nc.compile()
    return nc


def quantize_params(weight: np.ndarray, bias: np.ndarray):
    """Ternary-quantize weight/bias exactly as the reference (f64 math whose
    f32 rounding matches jax-f32; verified margins are orders of magnitude
    above f32 accumulation differences)."""
    w64 = weight.astype(np.float64)
    g_w = np.float32(np.abs(w64).mean())
    wi = np.clip(np.round(w64 / (np.float64(g_w) + EPS)), -1.0, 1.0)
    b64 = bias.astype(np.float64)
    g_b = np.float32(np.abs(b64).mean())
    bi = np.clip(np.round(b64 / (np.float64(g_b) + EPS)), -1.0, 1.0)
    bq = (bi * np.float64(g_b)).astype(np.float32)  # exact: {-g_b, 0, g_b}
    return wi, g_w, bq


_PROGRAM_CACHE: dict[int, bacc.Bacc] = {}


def _get_program(rows: int) -> bacc.Bacc:
    if rows not in _PROGRAM_CACHE:
        _PROGRAM_CACHE[rows] = build_program(rows)
    return _PROGRAM_CACHE[rows]


def tile_x_shard(x2d: np.ndarray) -> np.ndarray:
    """[rows, IN_F] -> [nt, P, KC, RSUB] with xt[t,p,c,r] = x[t*RSUB+r, c*P+p]."""
    rows = x2d.shape[0]
    return np.ascontiguousarray(
        x2d.reshape(rows // RSUB, RSUB, KC, P).transpose(0, 3, 2, 1)
    )


def prepare_in_maps(x: np.ndarray, weight: np.ndarray, bias: np.ndarray):
    x = np.asarray(x, dtype=np.float32)
    weight = np.asarray(weight, dtype=np.float32)
    bias = np.asarray(bias, dtype=np.float32)
    batch, rows, in_f = x.shape
    assert batch == N_CORES and in_f == IN_F and weight.shape == (OUT_F, IN_F)

    wi, g_w, bq = quantize_params(weight, bias)
    wq_t = np.ascontiguousarray(wi.T).astype(ml_dtypes.bfloat16)  # [in, out]
    bq_row = np.ascontiguousarray(bq.reshape(1, OUT_F))
    gq = np.array([[g_w]], dtype=np.float32)

    in_maps = []
    for c in range(N_CORES):
        in_maps.append(
            {
                "xt": tile_x_shard(x[c]),
                "wq": wq_t,
                "bq": bq_row,
                "gq": gq,
            }
        )
    return in_maps, rows


def kernel(x: np.ndarray, weight: np.ndarray, bias: np.ndarray) -> np.ndarray:
    in_maps, rows = prepare_in_maps(x, weight, bias)
    nc = _get_program(rows)
    res = bass_utils.run_bass_kernel_spmd(nc, in_maps, core_ids=list(range(N_CORES)))
    return np.stack([res.results[c]["y"] for c in range(N_CORES)], axis=0)


# revision 38
# speedup vs baseline: 1.3447x; 1.1829x over previous
"""BitNetLinear Trainium2 kernel (8 NeuronCores, SPMD data-parallel).

y = round(clip(x, +-127*s)/s)*s @ (ternary(W))^T + ternary(b)
with s = exp2(floor(log2(max|x|/127 + eps))) a power of two (global over x).

Sharding: batch dim (8) -> one batch of [4096, 1024] per core.
Host prep: each x shard is transposed/tiled to [nt, P, KC, RSUB] (the PE
contracts over partitions, and pre-tiling makes every device load fully
contiguous); weight/bias are ternary-quantized on host (the reference does
this once in __init__); the ternary weight ships as bf16 [in, out].

Device (single pass over x): each 1 MiB chunk of x^T is loaded once; an
absmax reduction and the quantize+matmul pipeline both read the same
resident tile. Because floor(log2(.)) commutes with max, the global
power-of-two scale is the max of per-core scales, and with overwhelming
probability it equals the scale of the first 262144-sample chunk
(P(mismatch) ~ e^-38 for randn data). So the kernel SPECULATES with
chunk-0's scale and starts the matmul pipeline at once; the full local max
and a 512B AllReduce(max) trail behind on the collectives hardware. At the
end each core checks speculated-scale == global-scale (one binade compare)
and, on mismatch, re-runs the exact quantize+matmul with the global scale
under a conditional branch - so the result is exact for ANY input.

x is quantized to integer-valued bf16 (round-half-even via the +-1.5*2^23
trick); the bf16 matmul with fp32 PSUM accumulation is exact integer
arithmetic (|x_int| <= 127, w in {-1,0,1}, |acc| < 2^24); the result is
scaled by s*gamma_w and the ternary bias is added.
"""

import numpy as np
import ml_dtypes
from contextlib import ExitStack

import concourse.bass as bass
import concourse.mybir as mybir
import concourse.tile as tile
from concourse import bacc, bass_isa, bass_utils

F32 = mybir.dt.float32
BF16 = mybir.dt.bfloat16
I32 = mybir.dt.int32

N_CORES = 8
P = 128
IN_F = 1024
OUT_F = 1024
KC = IN_F // P          # 8 contraction chunks
RSUB = 256              # rows per chunk
ROUND_C = 12582912.0    # 1.5 * 2**23: float32 round-half-even trick
EPS = 1e-8


def _emit_scale_chain(nc, consts, gmax, gamma_sb, mask_t, expc_t, tag):
    """From a [P,1] absmax tile, compute s = exp2(floor(log2(m/127+eps)))
    via exponent masking, 1/s via exponent arithmetic, and c = s*gamma."""
    v_t = consts.tile([P, 1], F32, tag=f"v_{tag}")
    nc.vector.tensor_scalar(
        out=v_t,
        in0=gmax,
        scalar1=float(np.float32(1.0 / 127.0)),
        scalar2=float(np.float32(EPS)),
        op0=mybir.AluOpType.mult,
        op1=mybir.AluOpType.add,
    )
    s_t = consts.tile([P, 1], F32, tag=f"s_{tag}")
    nc.vector.tensor_tensor(
        out=s_t.bitcast(I32),
        in0=v_t.bitcast(I32),
        in1=mask_t,
        op=mybir.AluOpType.bitwise_and,
    )
    inv_t = consts.tile([P, 1], F32, tag=f"inv_{tag}")
    nc.vector.tensor_tensor(
        out=inv_t.bitcast(I32),
        in0=expc_t,
        in1=s_t.bitcast(I32),
        op=mybir.AluOpType.subtract,
    )
    c_t = consts.tile([P, 1], F32, tag=f"c_{tag}")
    nc.vector.tensor_mul(out=c_t, in0=s_t, in1=gamma_sb)
    return s_t, inv_t, c_t


def _emit_phase2(nc, pools, rows, xt, y_rows, w_sb, bias_sb, negc_t,
                 scale, const_loader=None, chunk_hook=None):
    """Quantize x with 1/s, matmul against the ternary weight, scale by c,
    add bias, store y. `scale` is a dict read lazily (its "inv"/"c" tiles
    may be filled by chunk_hook at t==0). Returns the last emitted
    instruction per engine."""
    xq_pool, tq_pool, ub_pool, xi_pool, yo_pool, ps_pool = pools
    nhalf = OUT_F // 512
    last = {}
    for t in range(rows // RSUB):
        xc = xq_pool.tile([P, KC, RSUB], F32, tag="xc")
        last["SP"] = nc.sync.dma_start(out=xc, in_=xt[t])
        if t == 0 and const_loader is not None:
            const_loader()
        if chunk_hook is not None:
            chunk_hook(t, xc)
        tq = tq_pool.tile([P, KC, RSUB], F32, tag="tq")
        nc.vector.tensor_scalar(
            out=tq,
            in0=xc,
            scalar1=scale["inv"],
            scalar2=ROUND_C,
            op0=mybir.AluOpType.mult,
            op1=mybir.AluOpType.add,
        )
        ub = ub_pool.tile([P, KC, RSUB], BF16, tag="ub")
        nc.scalar.activation(
            out=ub,
            in_=tq,
            func=mybir.ActivationFunctionType.Identity,
            bias=negc_t,
            scale=1.0,
        )
        xi = xi_pool.tile([P, KC, RSUB], BF16, tag="xi")
        nc.vector.tensor_scalar(
            out=xi,
            in0=ub,
            scalar1=-127.0,
            scalar2=127.0,
            op0=mybir.AluOpType.max,
            op1=mybir.AluOpType.min,
        )
        for h in range(RSUB // P):
            ps = ps_pool.tile([P, OUT_F], F32, tag="ps")
            for k in range(KC):
                for n in range(nhalf):
                    last["PE"] = nc.tensor.matmul(
                        ps[:, n * 512 : (n + 1) * 512],
                        lhsT=xi[:, k, h * P : (h + 1) * P],
                        rhs=w_sb[:, k, n * 512 : (n + 1) * 512],
                        start=(k == 0),
                        stop=(k == KC - 1),
                    )
            yo = yo_pool.tile([P, OUT_F], F32, tag="yo")
            last["ACT"] = nc.scalar.activation(
                out=yo,
                in_=ps,
                func=mybir.ActivationFunctionType.Copy,
                bias=0.0,
                scale=scale["c"],
            )
            last["DVE"] = nc.vector.tensor_add(out=yo, in0=yo, in1=bias_sb)
            last["SP"] = nc.sync.dma_start(out=y_rows[t * (RSUB // P) + h], in_=yo)
    return last


def build_program(rows: int = 4096, num_cores: int = N_CORES) -> bacc.Bacc:
    assert rows % RSUB == 0
    nc = bacc.Bacc(
        "TRN2",
        target_bir_lowering=False,
        debug=False,
        enable_asserts=False,
        num_devices=num_cores,
    )
    nt = rows // RSUB
    # x shard pre-tiled on host: xt[t, p, c, r] = x[t*RSUB + r, c*P + p],
    # so every chunk load is one fully-contiguous 1 MiB read.
    xt = nc.dram_tensor("xt", (nt, P, KC, RSUB), F32, kind="ExternalInput").ap()
    wq = nc.dram_tensor("wq", (IN_F, OUT_F), BF16, kind="ExternalInput").ap()
    bq = nc.dram_tensor("bq", (1, OUT_F), F32, kind="ExternalInput").ap()
    gq = nc.dram_tensor("gq", (1, 1), F32, kind="ExternalInput").ap()
    y = nc.dram_tensor("y", (rows, OUT_F), F32, kind="ExternalOutput").ap()
    # Collectives cannot target I/O tensors; bounce through internal DRAM.
    cc_in = nc.dram_tensor("cc_in", (P, 1), F32).ap()
    cc_out = nc.dram_tensor("cc_out", (P, 1), F32).ap()

    with tile.TileContext(nc, num_cores=num_cores) as tc, ExitStack() as ctx:
        consts = ctx.enter_context(tc.tile_pool(name="consts", bufs=1))

        mask_t = consts.tile([P, 1], I32)
        nc.vector.memset(mask_t, -8388608)  # 0xFF800000: sign+exponent mask
        expc_t = consts.tile([P, 1], I32)
        nc.vector.memset(expc_t, 0x7F000000)  # bits of (254<<23)
        negc_t = consts.tile([P, 1], F32)
        nc.vector.memset(negc_t, -ROUND_C)

        # constants ride the ACT HWDGE ring, concurrent with the x stream
        gamma_sb = consts.tile([P, 1], F32)
        nc.scalar.dma_start(out=gamma_sb, in_=gq.to_broadcast((P, 1)))
        w_sb = consts.tile([P, KC, OUT_F], BF16)
        nc.scalar.dma_start(out=w_sb, in_=wq.rearrange("(c p) o -> p c o", p=P))
        bias_sb = consts.tile([P, OUT_F], F32)
        nc.scalar.dma_start(out=bias_sb, in_=bq.to_broadcast((P, OUT_F)))

        y_rows = y.rearrange("(t p) o -> t p o", p=P)
        partials = consts.tile([P, nt], F32)
        warm_rhs = consts.tile([P, 512], BF16)
        nc.vector.memset(warm_rhs, 0.0)

        scale_spec = {}
        gmax_l = consts.tile([P, 1], F32)
        gmax_g = consts.tile([P, 1], F32)
        twos_s = consts.tile([P, 1], F32)

        with (
            tc.tile_pool(name="xq", bufs=8) as xq_pool,
            tc.tile_pool(name="tq", bufs=2) as tq_pool,
            tc.tile_pool(name="ub", bufs=2) as ub_pool,
            tc.tile_pool(name="xi", bufs=3) as xi_pool,
            tc.tile_pool(name="yo", bufs=4) as yo_pool,
            tc.tile_pool(name="ps", bufs=4, space="PSUM") as ps_pool,
        ):
            pools = (xq_pool, tq_pool, ub_pool, xi_pool, yo_pool, ps_pool)

            def chunk_hook(t, xc):
                # fused absmax over the chunk just loaded (same read of x)
                nc.vector.tensor_reduce(
                    out=partials[:, t : t + 1],
                    in_=xc,
                    axis=mybir.AxisListType.XY,
                    op=mybir.AluOpType.max,
                    apply_absolute_value=True,
                )
                if t == 0:
                    # speculative scale from chunk 0 alone
                    lmax_s = consts.tile([P, 1], F32)
                    nc.vector.tensor_copy(out=lmax_s, in_=partials[:, 0:1])
                    gmax_s = consts.tile([P, 1], F32)
                    nc.gpsimd.partition_all_reduce(
                        gmax_s, lmax_s, channels=P,
                        reduce_op=bass_isa.ReduceOp.max,
                    )
                    s_s, inv_s, c_s = _emit_scale_chain(
                        nc, consts, gmax_s, gamma_sb, mask_t, expc_t, "l")
                    scale_spec["inv"] = inv_s
                    scale_spec["c"] = c_s
                    nc.vector.tensor_scalar(
                        out=twos_s, in0=s_s, scalar1=2.0, scalar2=None,
                        op0=mybir.AluOpType.mult,
                    )
                    # PE warmup: junk matmuls flip HAM to full clock while
                    # the first quantize runs
                    nc.vector.tensor_copy(out=warm_rhs[:, 0:1], in_=gmax_s)
                    warm_ps = ps_pool.tile([P, OUT_F], F32, tag="ps")
                    for _ in range(10):
                        nc.tensor.matmul(
                            warm_ps[:, 0:512], lhsT=warm_rhs[:, 0:P],
                            rhs=warm_rhs, start=True, stop=True,
                        )
                if t == nt - 1:
                    # full local max -> AllReduce(max) across the 8 cores,
                    # emitted here so the DVE/gpsimd FIFOs reach it as soon
                    # as the last chunk arrives (it trails under the matmul
                    # phase; only the final check waits on it)
                    lmax = consts.tile([P, 1], F32)
                    nc.vector.tensor_reduce(
                        out=lmax, in_=partials, axis=mybir.AxisListType.X,
                        op=mybir.AluOpType.max,
                    )
                    nc.gpsimd.partition_all_reduce(
                        gmax_l, lmax, channels=P,
                        reduce_op=bass_isa.ReduceOp.max,
                    )
                    nc.gpsimd.dma_start(out=cc_in, in_=gmax_l)
                    nc.gpsimd.collective_compute(
                        "AllReduce",
                        mybir.AluOpType.max,
                        replica_groups=[list(range(num_cores))],
                        ins=[cc_in.opt()],
                        outs=[cc_out.opt()],
                    )
                    nc.gpsimd.dma_start(out=gmax_g, in_=cc_out)

            last = _emit_phase2(
                nc, pools, rows, xt, y_rows, w_sb, bias_sb, negc_t,
                scale_spec, const_loader=None, chunk_hook=chunk_hook,
            )

            # --- verify the speculation: the speculative scale is the global
            # one iff v_g = gmax_g/127+eps stays in the same binade, i.e.
            # v_g - 2*s_spec < 0. Computed on the idle gpsimd engine. ---
            v_g2 = consts.tile([P, 1], F32)
            nc.gpsimd.tensor_scalar(
                out=v_g2,
                in0=gmax_g,
                scalar1=float(np.float32(1.0 / 127.0)),
                scalar2=float(np.float32(EPS)),
                op0=mybir.AluOpType.mult,
                op1=mybir.AluOpType.add,
            )
            chk = consts.tile([P, 1], F32)
            nc.gpsimd.tensor_tensor(
                out=chk, in0=v_g2, in1=twos_s, op=mybir.AluOpType.subtract
            )
            regs = nc.alloc_registers(
                "spec_chk",
                bass.OrderedSet([
                    mybir.EngineType.SP,
                    mybir.EngineType.DVE,
                    mybir.EngineType.Activation,
                    mybir.EngineType.PE,
                ]),
            )
            # Pin each engine's reg_load after its last speculative-phase
            # instruction: the load waits on the AllReduce, and the Tile
            # scheduler would otherwise be free to place it mid-stream,
            # stalling that engine's FIFO on the collective.
            eng_key = {
                mybir.EngineType.PE: "PE",
                mybir.EngineType.DVE: "DVE",
                mybir.EngineType.Activation: "ACT",
                mybir.EngineType.SP: "SP",
            }
            for reg in regs:
                ld = nc.reg_load(reg, chk.bitcast(I32)[0:1, 0:1])
                prev = last.get(eng_key[reg.engine])
                if prev is not None:
                    tile.add_dep_helper(
                        ld.ins, prev.ins, sync=False,
                        reason="speculation check after spec phase",
                    )
            # f32 bits as int32: negative iff chk < 0 (fast path)
            with tc.If(nc.snap(regs) >= 0):
                # mismatch: redo everything with the global scale
                _, inv_g, c_g = _emit_scale_chain(
                    nc, consts, gmax_g, gamma_sb, mask_t, expc_t, "g")
                _emit_phase2(nc, pools, rows, xt, y_rows, w_sb, bias_sb,
                             negc_t, {"inv": inv_g, "c": c_g})

    nc.compile()
    return nc


def quantize_params(weight: np.ndarray, bias: np.ndarray):
    """Ternary-quantize weight/bias exactly as the reference (f64 math whose
    f32 rounding matches jax-f32; verified margins are orders of magnitude
    above f32 accumulation differences)."""
    w64 = weight.astype(np.float64)
    g_w = np.float32(np.abs(w64).mean())
    wi = np.clip(np.round(w64 / (np.float64(g_w) + EPS)), -1.0, 1.0)
    b64 = bias.astype(np.float64)
    g_b = np.float32(np.abs(b64).mean())
    bi = np.clip(np.round(b64 / (np.float64(g_b) + EPS)), -1.0, 1.0)
    bq = (bi * np.float64(g_b)).astype(np.float32)  # exact: {-g_b, 0, g_b}
    return wi, g_w, bq


_PROGRAM_CACHE: dict[int, bacc.Bacc] = {}


def _get_program(rows: int) -> bacc.Bacc:
    if rows not in _PROGRAM_CACHE:
        _PROGRAM_CACHE[rows] = build_program(rows)
    return _PROGRAM_CACHE[rows]


def tile_x_shard(x2d: np.ndarray) -> np.ndarray:
    """[rows, IN_F] -> [nt, P, KC, RSUB] with xt[t,p,c,r] = x[t*RSUB+r, c*P+p]."""
    rows = x2d.shape[0]
    return np.ascontiguousarray(
        x2d.reshape(rows // RSUB, RSUB, KC, P).transpose(0, 3, 2, 1)
    )


def prepare_in_maps(x: np.ndarray, weight: np.ndarray, bias: np.ndarray):
    x = np.asarray(x, dtype=np.float32)
    weight = np.asarray(weight, dtype=np.float32)
    bias = np.asarray(bias, dtype=np.float32)
    batch, rows, in_f = x.shape
    assert batch == N_CORES and in_f == IN_F and weight.shape == (OUT_F, IN_F)

    wi, g_w, bq = quantize_params(weight, bias)
    wq_t = np.ascontiguousarray(wi.T).astype(ml_dtypes.bfloat16)  # [in, out]
    bq_row = np.ascontiguousarray(bq.reshape(1, OUT_F))
    gq = np.array([[g_w]], dtype=np.float32)

    in_maps = []
    for c in range(N_CORES):
        in_maps.append(
            {
                "xt": tile_x_shard(x[c]),
                "wq": wq_t,
                "bq": bq_row,
                "gq": gq,
            }
        )
    return in_maps, rows


def kernel(x: np.ndarray, weight: np.ndarray, bias: np.ndarray) -> np.ndarray:
    in_maps, rows = prepare_in_maps(x, weight, bias)
    nc = _get_program(rows)
    res = bass_utils.run_bass_kernel_spmd(nc, in_maps, core_ids=list(range(N_CORES)))
    return np.stack([res.results[c]["y"] for c in range(N_CORES)], axis=0)
